# revision 11
# baseline (speedup 1.0000x reference)
"""ACSL loss kernel for 8 TRN2 NeuronCores.

Strategy (data-parallel over N):
  Each core gets 2048 of the 16384 proposal rows. The reference loss

      L = sum_ij wm[i,j] * (softplus(x[i,j]) - x[i,j]*onehot[i,j]) / N

  (in permuted-column space) is decomposed so the device only does the
  O(N*C) work:

   - The column permutation is folded into the C-length class vectors on
     the host (roll by -1); one_hot(lab) in permuted space equals
     one_hot(labels) in original space.
   - wm at the own-label column is always 1, so the -x*onehot term is
     -sum_i x[i, labels[i]]  -> host gather, O(N).
   - fg rows: sum_j max(hs, onehot)*sp = sum_j hs*sp + (1-hs_lab)*sp_lab.
     The second piece is an O(N) host correction. The first piece uses
         hs*sp = relu(sp - t) + t*[x >= thr]      (t = softplus(thr))
     so two 4x-rate VectorE tensor_scalar ops with fused row-sum
     accumulation produce per-row partial sums; one TensorE matmul
     against the per-row fg flags reduces them (host takes the diagonal).
   - bg rows: the weight row is one of 4 vectors w(sr,sc) (host-computed,
     including the min(.,1) clip and the forced background column), so
         sum_{bg} sum_j w_k[j]*sp[i,j] = sum_k dot(w_k, colsum_k)
     where colsum_k = G_k^T @ sp is a 4-column TensorE matmul accumulated
     in PSUM across all row tiles. Host does the final dot.

  softplus itself is Ln(1*Exp(x) + 1) — two ScalarE passes sharing the
  natural_log_exp_and_others ACT table set (no softplus table exists in
  this toolchain; x ~ N(0,1) is bounded so Exp cannot overflow).

  Device pipeline per tile: DMA x (bf16) -> ScalarE Exp -> ScalarE Ln ->
  VectorE tensor_scalar x2 (accums) -> TensorE matmuls accumulating in
  PSUM. Inputs are cast to bf16 on the host (loss error ~4e-4, verified
  against the f64 reference decomposition).
"""

import sys

for _p in ("/opt/trn_rl_repo",):
    if _p not in sys.path:
        sys.path.insert(0, _p)

import numpy as np
from ml_dtypes import bfloat16

import concourse.bass as bass
import concourse.mybir as mybir
import concourse.tile as tile
from concourse.bass_utils import run_bass_kernel_spmd

N = 16384
C = 1204
NCORES = 8
ROWS_PER_CORE = N // NCORES          # 2048
P = 128                              # SBUF partitions
R = 4                                # row-slices per partition per supertile
NT = ROWS_PER_CORE // (P * R)        # supertiles per core
NH = NT * R                          # 128-row halves per core
THR = float(np.log(0.7 / 0.3))       # sigmoid(x) >= 0.7  <=>  x >= THR
T_SP = float(np.log(1.0 + 0.7 / 0.3))  # softplus(THR)
# column chunks for the bg matmul (PSUM bank = 512 fp32 per matmul)
CHUNKS = [(0, 512), (512, 512), (1024, C - 1024)]

_compiled = {}


def _split_waits(nc, max_waits=1):
    """Walrus codegen rejects instructions carrying more than one sem-wait
    ("Too many sync wait commands"); hoist extras onto single-wait NoOps on
    the same engine immediately before the instruction."""
    for fn in nc.m.functions:
        for blk in fn.blocks:
            out = []
            for inst in blk.instructions:
                si = inst.sync_info
                waits = list(si.on_wait) if si and si.on_wait else []
                if len(waits) > max_waits:
                    head, tail = waits[:-max_waits], waits[-max_waits:]
                    for j, w in enumerate(head):
                        out.append(mybir.InstNoOp(
                            name=f"{inst.name}-sw{j}",
                            engine=inst.engine,
                            ins=[], outs=[],
                            sync_info=mybir.SyncInfo(on_wait=[w],
                                                     on_update=[]),
                        ))
                    inst.sync_info = mybir.SyncInfo(
                        on_wait=tail, on_update=list(si.on_update or []))
                out.append(inst)
            blk.instructions = out


def _build_graph():
    nc = bass.Bass()
    x_d = nc.dram_tensor("x", [ROWS_PER_CORE, C], mybir.dt.bfloat16,
                         kind="ExternalInput")
    gb_d = nc.dram_tensor("gbg", [NT, P, R * 4], mybir.dt.bfloat16,
                          kind="ExternalInput")
    gf_d = nc.dram_tensor("gfg", [P, NH], mybir.dt.float32,
                          kind="ExternalInput")
    out_d = nc.dram_tensor("out", [4 + NH, C], mybir.dt.float32,
                           kind="ExternalOutput")

    x_view = x_d.rearrange("(s p r) c -> s p (r c)", s=NT, p=P, r=R)
    F = mybir.ActivationFunctionType

    with tile.TileContext(nc) as tc:
        with (
            tc.tile_pool(name="xin", bufs=4) as xpool,
            tc.tile_pool(name="e", bufs=2) as epool,
            tc.tile_pool(name="sp", bufs=4) as sppool,
            tc.tile_pool(name="scr", bufs=2) as scrpool,
            tc.tile_pool(name="small", bufs=1) as smpool,
            tc.tile_pool(name="g", bufs=2) as gpool,
            tc.tile_pool(name="ps", bufs=1, space="PSUM") as pspool,
        ):
            psum_bg = pspool.tile([4, C], mybir.dt.float32, tag="psbg")
            psum_fg = pspool.tile([NH, NH], mybir.dt.float32, tag="psfg")
            # per-half row-sum strips (tensor_scalar accum_out semantics:
            # accum = reduce(out, op1, init=scalar2); out = in0 op0 scalar1)
            maxstrip = smpool.tile([P, NH], mybir.dt.float32, tag="qs")
            cntstrip = smpool.tile([P, NH], mybir.dt.float32, tag="ms")
            gf = smpool.tile([P, NH], mybir.dt.float32, tag="gf")
            nc.sync.dma_start(gf[:], gf_d[:])

            for s in range(NT):
                xt = xpool.tile([P, R * C], mybir.dt.bfloat16)
                nc.sync.dma_start(xt[:], x_view[s])
                gb = gpool.tile([P, R * 4], mybir.dt.bfloat16, tag="gb")
                nc.sync.dma_start(gb[:], gb_d[s])

                et = epool.tile([P, R * C], mybir.dt.bfloat16, tag="e")
                nc.scalar.activation(et[:], xt[:], F.Exp)
                spt = sppool.tile([P, R * C], mybir.dt.bfloat16, tag="sp")
                nc.scalar.activation(spt[:], et[:], F.Ln, bias=1.0)

                for r in range(R):
                    h = s * R + r
                    sl = slice(r * C, (r + 1) * C)
                    # maxstrip[:,h] = sum_j max(sp, t)
                    #              = sum_j relu(sp - t) + t*C
                    sq = scrpool.tile([P, C], mybir.dt.bfloat16, tag="sq")
                    nc.vector.tensor_scalar(
                        out=sq[:], in0=spt[:, sl],
                        scalar1=T_SP, scalar2=0.0,
                        op0=mybir.AluOpType.max,
                        op1=mybir.AluOpType.add,
                        accum_out=maxstrip[:, h:h + 1],
                    )
                    # cntstrip[:,h] = count_j[x >= thr]
                    sm = scrpool.tile([P, C], mybir.dt.bfloat16, tag="sm")
                    nc.vector.tensor_scalar(
                        out=sm[:], in0=xt[:, sl],
                        scalar1=THR, scalar2=0.0,
                        op0=mybir.AluOpType.is_ge,
                        op1=mybir.AluOpType.add,
                        accum_out=cntstrip[:, h:h + 1],
                    )
                    for c0, cw in CHUNKS:
                        nc.tensor.matmul(
                            psum_bg[0:4, c0:c0 + cw],
                            lhsT=gb[:, r * 4:(r + 1) * 4],
                            rhs=spt[:, r * C + c0: r * C + c0 + cw],
                            start=(h == 0),
                            stop=(h == NH - 1),
                        )

            # fg reduction: comb = maxstrip + t*cntstrip (per-row
            # sum_j max(sp,t) + t*cnt = sum_j hs*sp + t*C, host removes
            # the t*C*n_fg constant); psum_fg = gf^T @ comb
            comb = smpool.tile([P, NH], mybir.dt.float32, tag="comb")
            nc.vector.scalar_tensor_tensor(
                out=comb[:], in0=cntstrip[:], scalar=T_SP, in1=maxstrip[:],
                op0=mybir.AluOpType.mult, op1=mybir.AluOpType.add)
            nc.tensor.matmul(psum_fg[:, :], lhsT=gf[:], rhs=comb[:],
                             start=True, stop=True)

            out_bg = smpool.tile([4, C], mybir.dt.float32, tag="obg")
            out_fg = smpool.tile([NH, NH], mybir.dt.float32, tag="ofg")
            nc.vector.tensor_copy(out_bg[:], psum_bg[:])
            nc.scalar.copy(out_fg[:], psum_fg[:])
            nc.sync.dma_start(out_d[0:4, :], out_bg[:])
            nc.sync.dma_start(out_d[4:4 + NH, 0:NH], out_fg[:])
    _split_waits(nc)
    return nc


def _get_graph():
    if "nc" not in _compiled:
        _compiled["nc"] = _build_graph()
    return _compiled["nc"]


def _prep(cls_logits, labels, rare_sel, common_sel, rare_vec, common_vec,
          freq_vec):
    """Host-side preprocessing. Returns (in_maps, W, host_const)."""
    x = np.asarray(cls_logits, np.float32)
    labels = np.asarray(labels).astype(np.int64)
    rare_sel = np.asarray(rare_sel).astype(bool)
    common_sel = np.asarray(common_sel).astype(bool)

    # class vectors rolled from permuted space to original column space
    rare_o = np.roll(np.asarray(rare_vec).astype(np.float64), -1)
    common_o = np.roll(np.asarray(common_vec).astype(np.float64), -1)
    freq_o = np.roll(np.asarray(freq_vec).astype(np.float64), -1)

    def wvec(sr, sc):
        w = np.minimum(freq_o + sr * rare_o + sc * common_o, 1.0)
        w[C - 1] = 1.0  # permuted col 0 (background) -> original col C-1
        return w

    W = np.stack([wvec(0, 0), wvec(1, 0), wvec(0, 1), wvec(1, 1)])  # k=sr+2sc

    is_bg = labels == C - 1
    fg = ~is_bg
    k = rare_sel.astype(np.int64) + 2 * common_sel.astype(np.int64)

    # host O(N) corrections (f64)
    g = x[np.arange(N), labels].astype(np.float64)
    own_term = -np.sum(g)
    g_hs = (g >= THR)
    fg_corr = float(np.sum((np.logaddexp(0.0, g) * (1.0 - g_hs))[fg]))
    # device fg diag carries a +t*C per fg row from the max(sp,t) identity
    host_const = own_term + fg_corr - T_SP * C * float(fg.sum())

    # per-row device flags
    G = np.zeros((N, 4), np.float32)
    G[np.arange(N)[is_bg], k[is_bg]] = 1.0
    fgf = fg.astype(np.float32)

    xb = x.astype(bfloat16)
    in_maps = []
    for c in range(NCORES):
        rows = slice(c * ROWS_PER_CORE, (c + 1) * ROWS_PER_CORE)
        # row = s*(P*R) + p*R + r; gfg[p, s*R+r] = fg flag of that row
        fgc = fgf[rows].reshape(NT, P, R)           # [s, p, r]
        gfg = np.ascontiguousarray(
            fgc.transpose(1, 0, 2).reshape(P, NH))  # [p, s*R+r]
        in_maps.append({
            "x": np.ascontiguousarray(xb[rows]),
            "gbg": np.ascontiguousarray(
                G[rows].reshape(NT, P, R * 4).astype(bfloat16)),
            "gfg": gfg,
        })
    return in_maps, W, host_const


def _reduce(results, W, host_const):
    total = host_const
    for res in results:
        out = np.asarray(res["out"], np.float64)
        total += float(np.sum(W * out[0:4]))
        total += float(np.trace(out[4:4 + NH, 0:NH]))
    return np.float32(total / N)


def kernel(cls_logits, labels, rare_sel, common_sel, rare_vec, common_vec,
           freq_vec, _run_kwargs=None):
    in_maps, W, host_const = _prep(cls_logits, labels, rare_sel, common_sel,
                                   rare_vec, common_vec, freq_vec)
    nc = _get_graph()
    kw = dict(_run_kwargs or {})
    res = run_bass_kernel_spmd(nc, in_maps, core_ids=list(range(NCORES)), **kw)
    out = _reduce(res.results, W, host_const)
    if kw:
        _compiled["last_results"] = res
    return out
